# revision 24
# baseline (speedup 1.0000x reference)
"""Trainium2 Bass kernel for nn_Bert_69698729280007.

Data-parallel over batch: core b processes batch row b (2 chunks of 512
tokens through the 4-layer BERT encoder), then does its own offset-based
segment mean-pool.  No collectives.

v3 layout/schedule:
- Residual stream TRANSPOSED [D, tokens]: fp32 carry (X32) plus bf16
  copies (Xb) used as GEMM inputs.  All GEMMs bf16 with fp32 PSUM.
- Both 512-token chunks processed per layer; emission is ZIPPERED so
  the ACT-bound softmax phases are covered by the other chunk's QKV/O
  matmuls (keeps the PE HAM-warm).
- Attention: V is stored augmented per head as [ones(64) | V_h(64)], so
  one [128,512]-out matmul chain per head yields softmax denominators
  (rows 0:64) AND unnormalized ctx (rows 64:128) in one pass — no
  separate denominator matmuls.  1/den via reciprocal_approx_fast; the
  normalize multiply reads the PSUM hi half directly.
- LayerNorm stats via ones-matmuls on bf16 copies; istd =
  Sqrt(recip_fast(var)); fp32 apply writes the fp32 carry + bf16 copy.
- FFN2 consumes H1 tiles k-outer into 6 persistent PSUM accumulators.
- Segment mean-pool via mask-matmul G[t, w] = (st_w <= t < ed_w).
"""

import os
import sys
from contextlib import ExitStack

import numpy as np
import ml_dtypes

for _p in ("/opt/trn_rl_repo", "/root/.axon_site/_ro/trn_rl_repo"):
    if os.path.isdir(_p) and _p not in sys.path:
        sys.path.append(_p)

import concourse.bass as bass
import concourse.tile as tile
from concourse import bacc, mybir
from concourse.bass_utils import run_bass_kernel_spmd
from concourse.masks import make_identity

AF = mybir.ActivationFunctionType
ALU = mybir.AluOpType
F32 = mybir.dt.float32
BF16 = mybir.dt.bfloat16
FP8 = mybir.dt.float8e4
I32 = mybir.dt.int32
DR = mybir.MatmulPerfMode.DoubleRow
WS = 64.0   # fp8 weight scale for Wq/Wk/Wv

B, S, W = 8, 1024, 512
D, H, F, L, V = 768, 12, 3072, 4, 28996
CH = 512
EPS = 1e-12
P = 128
DT = D // P          # 6 d-tiles
FT = F // P          # 24 f-tiles
NH = H // 2          # 6 head pairs
KT = CH // P         # 4 key tiles per chunk
DH = D // H          # 64

_COLS = dict(bq=(0, 6), bk=(6, 6), bv=(12, 6), bo=(18, 6), b1f=(24, 24),
             b2f=(48, 6), g1=(54, 6), b1=(60, 6), g2=(66, 6), b2=(72, 6))

N_CORES = 8
WS_HOST = 64.0   # must match kernel-side WS


def _col(sm, name, i):
    off, _n = _COLS[name]
    return sm[:, off + i:off + i + 1]


def build_kernel(ctx: ExitStack, tc: tile.TileContext, io: dict):
    nc = tc.nc

    consts = ctx.enter_context(tc.tile_pool(name="consts", bufs=1))
    big = ctx.enter_context(tc.tile_pool(name="big", bufs=1))
    psum = ctx.enter_context(tc.tile_pool(name="psum", bufs=1, space="PSUM"))

    # ---- constants ----
    ident_bf = consts.tile([P, P], BF16, tag="idbf")
    make_identity(nc, ident_bf)
    ident_f32 = consts.tile([P, P], F32, tag="idf32")
    make_identity(nc, ident_f32)
    ones_b = consts.tile([P, P], BF16, tag="onesb")
    nc.vector.memset(ones_b, 1.0)

    mask_sb = consts.tile([P, 8], F32, tag="masksb")
    nc.sync.dma_start(out=mask_sb, in_=io["mask128"])
    mb = consts.tile([P, 8], F32, tag="mb")
    nc.vector.tensor_scalar(mb, mask_sb, 10000.0, -10000.0,
                            op0=ALU.mult, op1=ALU.add)

    gb_emb = consts.tile([P, 2, D], BF16, tag="gbemb")
    nc.sync.dma_start(out=gb_emb, in_=io["emb_gb"][0:1, :, :].to_broadcast([P, 2, D]))
    pt_sb = consts.tile([P, 4, D], BF16, tag="ptsb")
    nc.sync.dma_start(out=pt_sb, in_=io["pos_type"].rearrange("(t p) d -> p t d", p=P))

    # final-h natural-layout tiles (bf16), persist until pooling
    h_nat = [big.tile([P, D], BF16, tag="hnat", bufs=8, name=f"hnat{t}")
             for t in range(8)]
    # augmented V tiles, persistent: [ones(64) | V_h(64)] per head
    vaug = {c: [big.tile([P, H, P], BF16, tag="vaug", bufs=8, name=f"vaug{c}{jk}")
                for jk in range(KT)] for c in (0, 1)}
    for c in (0, 1):
        for jk in range(KT):
            nc.vector.memset(vaug[c][jk][:, :, 0:64], 1.0)

    work_ctx = ExitStack()
    work = work_ctx.enter_context(tc.tile_pool(name="work", bufs=1))

    def mk_x8(X32n):
        """Pack fp32 tiles into fp8 k-pair tiles [P, 2, CH] for DoubleRow."""
        X8n = []
        for kp in range(DT // 2):
            x8t = work.tile([P, 2, CH], FP8, tag="x8", bufs=7, name="x8t")
            nc.vector.tensor_copy(x8t[:, 0, :], X32n[2 * kp])
            nc.vector.tensor_copy(x8t[:, 1, :], X32n[2 * kp + 1])
            X8n.append(x8t)
        return X8n

    def ln_txp(X1, sm, gname, bname):
        """LN over partition dim of transposed tiles.  X1: 6 fp32 pre-LN
        tiles (with residual).  Returns (X32new, Xbnew)."""
        ps1 = psum.tile([P, CH], F32, tag="mm", bufs=3, name="lnps1")
        ps2 = psum.tile([P, CH], F32, tag="mm", bufs=3, name="lnps2")
        for k in range(DT):
            xb16 = work.tile([P, CH], BF16, tag="xb16p", bufs=2, name="lnxb16")
            nc.vector.tensor_copy(xb16, X1[k])
            sq = work.tile([P, CH], BF16, tag="sq", bufs=2, name="lnsq")
            nc.scalar.activation(sq, xb16, AF.Square)
            nc.tensor.matmul(ps1, ones_b, xb16,
                             start=(k == 0), stop=(k == DT - 1))
            nc.tensor.matmul(ps2, ones_b, sq,
                             start=(k == 0), stop=(k == DT - 1))
        mean = work.tile([P, CH], BF16, tag="stat", bufs=3, name="lnmean")
        nc.scalar.activation(mean, ps1, AF.Copy, scale=1.0 / D)
        m2 = work.tile([P, CH], F32, tag="statf", bufs=2, name="lnm2")
        nc.vector.tensor_mul(m2, mean, mean)
        var = work.tile([P, CH], F32, tag="statf", bufs=2, name="lnvar")
        nc.vector.scalar_tensor_tensor(var, ps2, 1.0 / D, m2,
                                       op0=ALU.mult, op1=ALU.subtract)
        rv = work.tile([P, CH], F32, tag="statf", bufs=2, name="lnrv")
        nc.vector.reciprocal_approx_fast(rv, var)
        istd = work.tile([P, CH], BF16, tag="stat", bufs=3, name="lnistd")
        nc.scalar.activation(istd, rv, AF.Sqrt)
        X32n, Xbn = [], []
        for k in range(DT):
            t = work.tile([P, CH], F32, tag="lnt", bufs=2, name="lnt")
            nc.vector.tensor_sub(t, X1[k], mean)
            nc.vector.tensor_mul(t, t, istd)
            x32 = work.tile([P, CH], F32, tag="x32", bufs=13, name="lnx32")
            nc.vector.tensor_scalar(x32, t, _col(sm, gname, k),
                                    _col(sm, bname, k), op0=ALU.mult, op1=ALU.add)
            xbn = work.tile([P, CH], BF16, tag="xb", bufs=13, name="lnxb")
            nc.vector.tensor_copy(xbn, x32)
            X32n.append(x32)
            Xbn.append(xbn)
        return X32n, Xbn

    def embed_chunk(c):
        X32 = [work.tile([P, CH], F32, tag="x32", bufs=13, name=f"embx{k}")
               for k in range(DT)]
        for tt in range(KT):
            ids_sb = work.tile([P, 1], I32, tag="ids", bufs=2, name="idssb")
            nc.sync.dma_start(out=ids_sb, in_=io["ids"][c * 4 + tt])
            eg = work.tile([P, D], F32, tag="embg", bufs=2, name="embg")
            nc.gpsimd.indirect_dma_start(
                out=eg, out_offset=None, in_=io["word_emb"][:],
                in_offset=bass.IndirectOffsetOnAxis(ap=ids_sb[:, :1], axis=0))
            nc.vector.tensor_add(eg, eg, pt_sb[:, tt, :])
            stats = work.tile([P, 3, 6], F32, tag="bnst", bufs=2, name="bnst")
            egr = eg.rearrange("p (s q) -> p s q", s=3)
            for s in range(3):
                nc.vector.bn_stats(out=stats[:, s, :], in_=egr[:, s, :])
            mv = work.tile([P, 2], F32, tag="bnmv", bufs=2, name="bnmv")
            nc.vector.bn_aggr(out=mv, in_=stats)
            istd0 = work.tile([P, 1], F32, tag="bnis", bufs=2, name="bnis")
            nc.vector.tensor_scalar_add(istd0, mv[:, 1:2], EPS)
            nc.scalar.activation(istd0, istd0, AF.Sqrt)
            nc.vector.reciprocal(istd0, istd0)
            nc.vector.tensor_scalar(eg, eg, mv[:, 0:1], istd0,
                                    op0=ALU.subtract, op1=ALU.mult)
            nc.vector.tensor_mul(eg, eg, gb_emb[:, 0, :])
            nc.vector.tensor_add(eg, eg, gb_emb[:, 1, :])
            for k in range(DT):
                pt = psum.tile([P, P], F32, tag="sc", bufs=3, name="embtp")
                nc.tensor.transpose(pt, eg[:, k * P:(k + 1) * P], ident_f32)
                nc.vector.tensor_copy(X32[k][:, tt * P:(tt + 1) * P], pt)
        Xb = []
        for k in range(DT):
            xbn = work.tile([P, CH], BF16, tag="xb", bufs=13, name="embxb")
            nc.vector.tensor_copy(xbn, X32[k])
            Xb.append(xbn)
        return X32, Xb

    e0 = embed_chunk(0)
    e1 = embed_chunk(1)
    X32 = {0: e0[0], 1: e1[0]}
    Xb = {0: e0[1], 1: e1[1]}
    X8 = {0: mk_x8(e0[0]), 1: mk_x8(e1[0])}

    # ================= encoder layers =================
    # ln2(c1) of layer l is deferred into layer l+1's emission so its
    # stats chain is covered by the next layer's Q-projection matmuls.
    pending_ln2 = None
    for l in range(L):
        sm = work.tile([P, 78], F32, tag="smalls", bufs=2, name="smalls")
        nc.sync.dma_start(out=sm, in_=io["smalls"][l])

        QT = {0: [None] * DT, 1: [None] * DT}
        KTt = {0: [None] * DT, 1: [None] * DT}
        ctxT = {0: [None] * NH, 1: [None] * NH}

        def qk_block(c, wkey, bn, dst, m):
            wsl = work.tile([P, DT, P], FP8, tag="wqk", bufs=6, name="wqksl")
            nc.sync.dma_start(out=wsl, in_=io[wkey][l, m])
            ps = psum.tile([P, CH], F32, tag="mm", bufs=3, name="qkps")
            for kp in range(DT // 2):
                nc.tensor.matmul(ps, wsl[:, 2 * kp:2 * kp + 2, :], X8[c][kp],
                                 start=(kp == 0), stop=(kp == DT // 2 - 1),
                                 perf_mode=DR)
            o = work.tile([P, CH], BF16, tag="qk", bufs=18, name="qkt")
            nc.vector.tensor_scalar(o, ps, 1.0 / WS, _col(sm, bn, m),
                                    op0=ALU.mult, op1=ALU.add)
            dst[c][m] = o

        def v_block(c, nn):
            wvs = []
            for kp in range(DT // 2):
                wv = work.tile([P, 2, 384], FP8, tag="wv", bufs=3, name="wvsl")
                nc.sync.dma_start(out=wv, in_=io["Wv"][l, kp, nn])
                wvs.append(wv)
            for mt in range(KT):
                ps = psum.tile([P, 384], F32, tag="mm", bufs=3, name="vps")
                for kp in range(DT // 2):
                    nc.tensor.matmul(ps, X8[c][kp][:, :, mt * P:(mt + 1) * P],
                                     wvs[kp], start=(kp == 0),
                                     stop=(kp == DT // 2 - 1), perf_mode=DR)
                nc.vector.tensor_scalar(
                    vaug[c][mt][:, 6 * nn:6 * nn + 6, 64:128],
                    ps.rearrange("p (h d) -> p h d", h=6), 1.0 / WS, None,
                    op0=ALU.mult)

        Et = {}

        def scores(c, p):
            Et[(c, p)] = {0: [], 1: []}
            for jk in range(KT):
                for hh in (0, 1):
                    lo = hh * 64
                    ps = psum.tile([P, CH], F32, tag="sc", bufs=3, name="scps")
                    nc.tensor.matmul(
                        ps, KTt[c][p][lo:lo + 64, jk * P:(jk + 1) * P],
                        QT[c][p][lo:lo + 64, :], start=True, stop=True)
                    e = work.tile([P, CH], BF16, tag="e", bufs=9, name="etile")
                    nc.scalar.activation(
                        e, ps, AF.Exp, scale=0.125,
                        bias=mb[:, c * 4 + jk: c * 4 + jk + 1])
                    Et[(c, p)][hh].append(e)

        def augctx(c, p):
            cx = work.tile([P, CH], BF16, tag="ctx", bufs=11, name="ctxt")
            for hh in (0, 1):
                h = 2 * p + hh
                pch = psum.tile([P, CH], F32, tag="cx", bufs=2, name="augps")
                for jk in range(KT):
                    nc.tensor.matmul(pch, vaug[c][jk][:, h, :],
                                     Et[(c, p)][hh][jk],
                                     start=(jk == 0), stop=(jk == KT - 1))
                rec = work.tile([64, CH], F32, tag="rd", bufs=2, name="recd")
                nc.vector.reciprocal_approx_fast(rec, pch[0:64, :])
                nc.vector.tensor_mul(cx[hh * 64:(hh + 1) * 64, :],
                                     pch[64:128, :], rec)
            nc.vector.tensor_scalar_add(cx, cx, _col(sm, "bv", p))
            ctxT[c][p] = cx
            del Et[(c, p)]

        X1 = {0: [None] * DT, 1: [None] * DT}

        def o_block(c, m):
            wsl = work.tile([P, DT, P], BF16, tag="wqk", bufs=6, name="wosl")
            nc.sync.dma_start(out=wsl, in_=io["Wo"][l, m])
            ps = psum.tile([P, CH], F32, tag="mm", bufs=3, name="ops")
            for kp in range(DT):
                nc.tensor.matmul(ps, wsl[:, kp, :], ctxT[c][kp],
                                 start=(kp == 0), stop=(kp == DT - 1))
            xp = work.tile([P, CH], F32, tag="pre", bufs=7, name="x1pre")
            nc.vector.scalar_tensor_tensor(xp, ps, _col(sm, "bo", m),
                                           X32[c][m], op0=ALU.add, op1=ALU.add)
            X1[c][m] = xp

        # ---- phase A: QKV(c0); deferred ln2(c1) hidden behind Q(c0) ----
        for m in range(DT):
            qk_block(0, "Wq", "bq", QT, m)
        if pending_ln2 is not None:
            pending_ln2()
            pending_ln2 = None
        for m in range(DT):
            qk_block(0, "Wk", "bk", KTt, m)
        for nn in range(2):
            v_block(0, nn)

        # ---- phase B: attn(c0) zippered with QKV(c1) ----
        fillB = ([lambda m=m: qk_block(1, "Wq", "bq", QT, m) for m in range(DT)]
                 + [lambda m=m: qk_block(1, "Wk", "bk", KTt, m) for m in range(DT)]
                 + [lambda nn=nn: v_block(1, nn) for nn in range(2)])
        fi = 0
        for p in range(NH):
            scores(0, p)
            for _ in range(2):
                if fi < len(fillB):
                    fillB[fi]()
                    fi += 1
            if p >= 1:
                augctx(0, p - 1)
        while fi < len(fillB):
            fillB[fi]()
            fi += 1
        augctx(0, NH - 1)

        # ---- phase C: attn(c1) zippered with O(c0) ----
        fi = 0
        for p in range(NH):
            scores(1, p)
            if fi < DT:
                o_block(0, fi)
                fi += 1
            if p >= 1:
                augctx(1, p - 1)
        while fi < DT:
            o_block(0, fi)
            fi += 1
        augctx(1, NH - 1)

        o_block(1, 0)
        o_block(1, 1)
        X32[0], Xb[0] = ln_txp(X1[0], sm, "g1", "b1")
        for m in range(2, DT):
            o_block(1, m)
        X32[1], Xb[1] = ln_txp(X1[1], sm, "g1", "b1")

        # ---- phase D: FFN + LN2 per chunk; FFN2 k-outer ----
        for c in (0, 1):
            # keep 2 of 3 "mm" slots free so the next layer's Q-projection
            # psums aren't starved while the stt chain drains the facc banks
            facc = [psum.tile([P, CH], F32, tag=t, bufs=b, name=f"f2acc{m}")
                    for m, (t, b) in enumerate(
                        (("sc", 3), ("sc", 3), ("sc", 3),
                         ("cx", 2), ("cx", 2), ("mm", 3)))]
            H1 = []
            w2_sb = {}

            def ffn2_group(k):
                for m in range(DT):
                    nc.tensor.matmul(facc[m], w2_sb[k % 4][:, m * P:(m + 1) * P],
                                     H1[k], start=(k == 0), stop=(k == FT - 1))

            for mg in range(DT):
                w1_sb = work.tile([P, DT, CH], BF16, tag="w1", bufs=2,
                                  name="w1sb")
                nc.sync.dma_start(out=w1_sb, in_=io["W1"][l, mg])
                for mm2 in range(4):
                    k = mg * 4 + mm2
                    w2_sb[k % 4] = work.tile([P, D], BF16, tag="w2", bufs=3,
                                             name="w2sb")
                    nc.sync.dma_start(out=w2_sb[k % 4], in_=io["W2"][l, k])
                    ps = psum.tile([P, CH], F32, tag="mm", bufs=3, name="f1ps")
                    for kk in range(DT):
                        nc.tensor.matmul(
                            ps, w1_sb[:, kk, mm2 * P:(mm2 + 1) * P],
                            Xb[c][kk], start=(kk == 0), stop=(kk == DT - 1))
                    h1t = work.tile([P, CH], BF16, tag="h1", bufs=6, name="h1t")
                    nc.scalar.activation(h1t, ps, AF.Gelu,
                                         bias=_col(sm, "b1f", k))
                    H1.append(h1t)
                    if k > 0:
                        ffn2_group(k - 1)
            ffn2_group(FT - 1)
            X2 = [None] * DT
            for m in (5, 0, 1, 2, 3, 4):    # release the mm-tag bank first
                xp = work.tile([P, CH], F32, tag="pre", bufs=7, name="x2pre")
                nc.vector.scalar_tensor_tensor(xp, facc[m], _col(sm, "b2f", m),
                                               X32[c][m], op0=ALU.add,
                                               op1=ALU.add)
                X2[m] = xp
            if c == 0:
                X32[0], Xb[0] = ln_txp(X2, sm, "g2", "b2")
                X8[0] = mk_x8(X32[0])
            else:
                def make_pending(X2c, smc):
                    def go():
                        X32[1], Xb[1] = ln_txp(X2c, smc, "g2", "b2")
                        X8[1] = mk_x8(X32[1])
                    return go
                pending_ln2 = make_pending(X2, sm)

    # ---- transpose final h back to natural layout (bf16) ----
    def final_transpose(c):
        for k in range(DT):
            for tt in range(KT):
                pt = psum.tile([P, P], BF16, tag="sc", bufs=3, name="fintp")
                nc.tensor.transpose(pt, Xb[c][k][:, tt * P:(tt + 1) * P],
                                    ident_bf)
                nc.vector.tensor_copy(h_nat[c * 4 + tt][:, k * P:(k + 1) * P],
                                      pt)

    final_transpose(0)
    pending_ln2()
    final_transpose(1)

    # ================= segment mean-pool =================
    work_ctx.close()
    work = ctx.enter_context(tc.tile_pool(name="poolph", bufs=1))
    stb = work.tile([P, W], F32, tag="stb", bufs=1, name="stb")
    nc.sync.dma_start(out=stb, in_=io["st_row"][0:1, :].to_broadcast([P, W]))
    edb = work.tile([P, W], F32, tag="edb", bufs=1, name="edb")
    nc.sync.dma_start(out=edb, in_=io["ed_row"][0:1, :].to_broadcast([P, W]))

    Gt = []
    for t in range(8):
        it = work.tile([P, 1], F32, tag="iota", bufs=2, name="iotat")
        nc.sync.dma_start(out=it, in_=io["iota8"][t])
        g = work.tile([P, W], BF16, tag="g", bufs=8, name="gtile")
        nc.vector.tensor_scalar(g, stb, it, None, op0=ALU.is_le)
        g2 = work.tile([P, W], BF16, tag="g2", bufs=2, name="g2tile")
        nc.vector.tensor_scalar(g2, edb, it, None, op0=ALU.is_gt)
        nc.vector.tensor_mul(g, g, g2)
        Gt.append(g)

    stp = work.tile([P, 4], F32, tag="stp", bufs=1, name="stp")
    nc.sync.dma_start(out=stp, in_=io["stp"])
    edp = work.tile([P, 4], F32, tag="edp", bufs=1, name="edp")
    nc.sync.dma_start(out=edp, in_=io["edp"])
    xmp = work.tile([P, 4], F32, tag="xmp", bufs=1, name="xmp")
    nc.sync.dma_start(out=xmp, in_=io["xmp"])
    rmask = work.tile([P, 4], F32, tag="rmask", bufs=1, name="rmask")
    nc.vector.tensor_sub(rmask, edp, stp)
    nc.vector.tensor_scalar_max(rmask, rmask, 1.0)
    nc.vector.reciprocal(rmask, rmask)
    t1 = work.tile([P, 4], F32, tag="pt1", bufs=1, name="pt1")
    nc.vector.tensor_scalar(t1, xmp, 0.0, None, op0=ALU.not_equal)
    nc.vector.tensor_mul(rmask, rmask, t1)
    nc.vector.tensor_tensor(t1, stp, edp, op=ALU.is_lt)
    nc.vector.tensor_mul(rmask, rmask, t1)

    for w in range(4):
        for dn in range(2):
            ps = psum.tile([P, 384], F32, tag="mm", bufs=3, name="poolps")
            for t in range(8):
                nc.tensor.matmul(ps, Gt[t][:, w * P:(w + 1) * P],
                                 h_nat[t][:, dn * 384:(dn + 1) * 384],
                                 start=(t == 0), stop=(t == 7))
            o = work.tile([P, 384], F32, tag="poolo", bufs=2, name="poolo")
            nc.scalar.activation(o, ps, AF.Copy, scale=rmask[:, w:w + 1])
            nc.sync.dma_start(
                out=io["out"][w * P:(w + 1) * P, dn * 384:(dn + 1) * 384], in_=o)


def build_program():
    nc = bacc.Bacc("TRN2", target_bir_lowering=False, debug=False,
                   num_devices=N_CORES)
    io = {}

    def inp(name, shape, dt):
        io[name] = nc.dram_tensor(name, list(shape), dt, kind="ExternalInput").ap()

    inp("ids", (8, P, 1), I32)
    inp("mask128", (P, 8), F32)
    inp("st_row", (1, W), F32)
    inp("ed_row", (1, W), F32)
    inp("stp", (P, 4), F32)
    inp("edp", (P, 4), F32)
    inp("xmp", (P, 4), F32)
    inp("iota8", (8, P, 1), F32)
    inp("word_emb", (V, D), F32)
    inp("pos_type", (CH, D), BF16)
    inp("emb_gb", (1, 2, D), BF16)
    inp("smalls", (L, P, 78), F32)
    inp("Wq", (L, DT, P, DT, P), FP8)
    inp("Wk", (L, DT, P, DT, P), FP8)
    inp("Wo", (L, DT, P, DT, P), BF16)
    inp("Wv", (L, DT // 2, 2, P, 2, 384), FP8)
    inp("W1", (L, DT, P, DT, CH), BF16)
    inp("W2", (L, FT, P, D), BF16)
    io["out"] = nc.dram_tensor("out", [W, D], F32, kind="ExternalOutput").ap()

    with tile.TileContext(nc) as tc:
        with ExitStack() as ctx:
            build_kernel(ctx, tc, io)
    nc.compile()
    return nc


_NC_CACHE = None


def _get_program():
    global _NC_CACHE
    if _NC_CACHE is None:
        _NC_CACHE = build_program()
    return _NC_CACHE


def make_in_maps(inputs):
    """Host-side prep: shard per batch row, reshape/cast into device layouts."""
    bf = ml_dtypes.bfloat16
    x_bert = np.asarray(inputs["x_bert"])
    x_mask_tok = np.asarray(inputs["x_bert_mask"], dtype=np.float32)
    off = np.asarray(inputs["x_bert_offset"])
    xm = np.asarray(inputs["x_mask"])
    word_emb = np.ascontiguousarray(np.asarray(inputs["word_emb"], np.float32))
    pos_type = np.asarray(inputs["pos_emb"], np.float32) + \
        np.asarray(inputs["type_emb"], np.float32)[0][None, :]
    pos_type = np.ascontiguousarray(pos_type.astype(bf))
    emb_gb = np.stack([np.asarray(inputs["emb_g"], np.float32),
                       np.asarray(inputs["emb_b"], np.float32)])[None]
    emb_gb = np.ascontiguousarray(emb_gb.astype(bf))

    smalls = np.zeros((L, P, 78), np.float32)
    for nm, key in (("bq", "bq"), ("bk", "bk"), ("bv", "bv"), ("bo", "bo"),
                    ("b1f", "b1f"), ("b2f", "b2f"), ("g1", "ln1_g"),
                    ("b1", "ln1_b"), ("g2", "ln2_g"), ("b2", "ln2_b")):
        offc, n = _COLS[nm]
        arr = np.asarray(inputs[key], np.float32)
        smalls[:, :, offc:offc + n] = arr.reshape(L, n, P).transpose(0, 2, 1)

    f8 = ml_dtypes.float8_e4m3
    wts = {}
    for k in ("Wq", "Wk"):
        w = np.asarray(inputs[k], np.float32) * WS_HOST    # [L, D, D]
        wts[k] = np.ascontiguousarray(
            w.reshape(L, DT, P, DT, P).transpose(0, 3, 2, 1, 4).astype(f8))
    wo = np.asarray(inputs["Wo"], np.float32).astype(bf)
    wts["Wo"] = np.ascontiguousarray(
        wo.reshape(L, DT, P, DT, P).transpose(0, 3, 2, 1, 4))  # [L,m,P,k,P]
    wv = np.asarray(inputs["Wv"], np.float32) * WS_HOST
    # [L, kp, j, P, nn, 384] -> [L, kp, nn, P, j, 384]
    wts["Wv"] = np.ascontiguousarray(
        wv.reshape(L, DT // 2, 2, P, 2, 384).transpose(0, 1, 4, 3, 2, 5)
        .astype(f8))
    w1 = np.asarray(inputs["W1"], np.float32).astype(bf)
    wts["W1"] = np.ascontiguousarray(
        w1.reshape(L, DT, P, DT, CH).transpose(0, 3, 2, 1, 4))   # [L,mg,P,k,CH]
    w2 = np.asarray(inputs["W2"], np.float32).astype(bf)
    wts["W2"] = np.ascontiguousarray(w2.reshape(L, FT, P, D))    # [L,k,P,D]

    iota8 = np.arange(S, dtype=np.float32).reshape(8, P, 1)

    in_maps = []
    for b in range(N_CORES):
        ids = np.ascontiguousarray(
            x_bert[b].astype(np.int32).reshape(8, P, 1))
        mask128 = np.ascontiguousarray(
            x_mask_tok[b].reshape(8, P).T.astype(np.float32))
        st = off[b, :, 0].astype(np.float32)
        ed = off[b, :, 1].astype(np.float32)
        m = {
            "ids": ids,
            "mask128": mask128,
            "st_row": st[None, :].copy(),
            "ed_row": ed[None, :].copy(),
            "stp": np.ascontiguousarray(st.reshape(4, P).T),
            "edp": np.ascontiguousarray(ed.reshape(4, P).T),
            "xmp": np.ascontiguousarray(
                xm[b].astype(np.float32).reshape(4, P).T),
            "iota8": iota8,
            "word_emb": word_emb,
            "pos_type": pos_type,
            "emb_gb": emb_gb,
            "smalls": smalls,
        }
        m.update(wts)
        in_maps.append(m)
    return in_maps


def kernel(**inputs):
    nc = _get_program()
    in_maps = make_in_maps(inputs)
    res = run_bass_kernel_spmd(nc, in_maps, list(range(N_CORES)))
    return np.stack([res.results[b]["out"] for b in range(N_CORES)])
